# revision 1
# baseline (speedup 1.0000x reference)
"""Trainium2 Bass kernel for the Cocoa contrastive loss.

loss = mean_i exp((1 - cos(x_i, y_i))/tau)
     + sum_{i in neg, j not in neg} exp(cos(x_i, x_j)/tau) / cnt
     + sum_{i in neg, j not in neg} exp(cos(y_i, y_j)/tau) / cnt

with neg = rows whose label has > 32 zeros, cnt = n_neg * n_nonneg.

Strategy (8 NeuronCores):
  Host: compute the neg mask (exact integer math), permute rows so neg rows
        come first, zero-pad the two groups to SPMD-friendly sizes.
  Phase 1 (data-parallel over 512 rows/core): row norms, normalize+cast to
        bf16, per-row cos(x_i,y_i) dots (pos term), PE-transpose the
        normalized embeddings into [D, rows] layout for the Gram GEMM.
        x is processed fully before y so x's transposes overlap y's loads.
  Phase 2 (4x2 grid over neg x nonneg): bf16 GEMM sim = A_neg @ B_nonneg^T
        with K=D on partitions, exp(sim/tau) on ScalarE with per-partition
        accumulation; returns [128, n_tiles] partial sums per core.
  Host: combine partial sums (subtract the exp(0)=1 contributions of the
        zero padding), compute pos term from the cos values in float64.
"""

import numpy as np
import ml_dtypes

import concourse.bass as bass
import concourse.bacc as bacc
import concourse.mybir as mybir
import concourse.tile as tile
from concourse.bass_utils import run_bass_kernel_spmd
from concourse.masks import make_identity

TAU = 0.1
THRESHOLD = 32
B, D, L = 4096, 4096, 64
NCORES = 8
ROWS = B // NCORES  # 512 rows per core in phase 1
KCH = D // 128      # 32 contraction chunks
A_SPLIT, B_SPLIT = 4, 2  # phase-2 core grid over (neg rows, nonneg rows)
NSUB = 384          # phase-2 N subtile (PSUM free dim)

F32 = mybir.dt.float32
BF16 = mybir.dt.bfloat16
FP8 = mybir.dt.float8e4
BF16_NP = ml_dtypes.bfloat16
FP8_NP = ml_dtypes.float8_e4m3fn
FP8_SCALE = 24.0  # centers N(0, 1/4096) values in e4m3's normal range

# module-level caches so repeated kernel() calls don't rebuild/recompile
_CACHE: dict = {}

# filled in by the last kernel() call when tracing is enabled (test harness use)
LAST_RESULTS: list = []


def _build_phase1() -> bass.Bass:
    nc = bacc.Bacc(None)
    x_in = nc.declare_dram_parameter("x", [ROWS, D], F32, isOutput=False)
    y_in = nc.declare_dram_parameter("y", [ROWS, D], F32, isOutput=False)
    xt_out = nc.declare_dram_parameter("xt", [KCH, 128, ROWS], BF16, isOutput=True)
    yt_out = nc.declare_dram_parameter("yt", [KCH, 128, ROWS], BF16, isOutput=True)
    # per-row [cos, ssx, ssy] for the host-side pos term
    dots_out = nc.declare_dram_parameter("dots", [128, ROWS // 128, 3], F32, isOutput=True)

    ngrp = ROWS // 128  # 4 row groups per core

    with tile.TileContext(nc) as tc:
        with (
            tc.tile_pool(name="inp", bufs=5) as inp,
            tc.tile_pool(name="big", bufs=1) as big,
            tc.tile_pool(name="junk", bufs=2) as junkp,
            tc.tile_pool(name="prodp", bufs=2) as prodp,
            tc.tile_pool(name="small", bufs=1) as small,
            tc.tile_pool(name="tpsum", bufs=4, space="PSUM") as tpsum,
            tc.tile_pool(name="tout", bufs=6) as tout,
        ):
            ident = small.tile([128, 128], BF16)
            make_identity(nc, ident)

            xn = big.tile([128, ngrp, D], BF16)
            yn = big.tile([128, ngrp, D], BF16)
            stats = small.tile([128, ngrp, 3], F32)  # [cos, ssx, ssy]

            # stage A (both tensors): loads + row sumsq + normalize-cast.
            # Emitting y's loads before x's stores keeps the sync engine from
            # serializing y's input DMAs behind 8MB of x output traffic.
            for t_idx, (src_dram, tn) in enumerate(((x_in, xn), (y_in, yn))):
                tiles = []
                for g in range(ngrp):
                    tg = inp.tile([128, D], F32, tag="ld")
                    nc.sync.dma_start(out=tg, in_=src_dram[g * 128:(g + 1) * 128, :])
                    tiles.append(tg)
                    jx = junkp.tile([128, D], BF16, tag="junk")
                    nc.scalar.activation(jx, tg,
                                         mybir.ActivationFunctionType.Square,
                                         accum_out=stats[:, g, 1 + t_idx:2 + t_idx])
                invs = []
                for g in range(ngrp):
                    inv = small.tile([128, 1], F32, tag=f"inv{t_idx}{g}")
                    nc.scalar.sqrt(inv, stats[:, g, 1 + t_idx:2 + t_idx])
                    nc.vector.reciprocal(inv, inv)
                    invs.append(inv)
                for g in range(ngrp):
                    nc.vector.tensor_scalar_mul(tn[:, g, :], tiles[g],
                                                invs[g][:, 0:1])

            # stage B: per-row cos (pos term) from the normalized bf16 tiles;
            # both DVE-written, so each TensorTensor needs a single sync wait.
            for g in range(ngrp):
                prod = prodp.tile([128, D], BF16, tag="prod")
                nc.vector.tensor_mul(prod, xn[:, g, :], yn[:, g, :])
                jd = junkp.tile([128, D], BF16, tag="junk")
                nc.scalar.activation(jd, prod,
                                     mybir.ActivationFunctionType.Copy,
                                     accum_out=stats[:, g, 0:1])

            # stage C: PE transposes, PSUM->SBUF copies (uint32 bitcast),
            # stores
            for tn, dst in ((xn, xt_out), (yn, yt_out)):
                for c in range(0, KCH, 2):
                    ps = tpsum.tile([128, 2, ROWS], BF16, tag="tp")
                    for cc in range(2):
                        for g in range(ngrp):
                            nc.tensor.transpose(
                                ps[:, cc, g * 128:(g + 1) * 128],
                                tn[:, g, (c + cc) * 128:(c + cc + 1) * 128],
                                ident)
                    sb = tout.tile([128, 2, ROWS], BF16, tag="to")
                    nc.vector.tensor_copy(sb.bitcast(mybir.dt.uint32),
                                          ps.bitcast(mybir.dt.uint32))
                    nc.sync.dma_start(out=dst[c, :, :], in_=sb[:, 0, :])
                    nc.sync.dma_start(out=dst[c + 1, :, :], in_=sb[:, 1, :])

            nc.sync.dma_start(out=dots_out[:], in_=stats)
    nc.compile()
    return nc


def _build_phase2(m_loc: int, n_loc: int) -> bass.Bass:
    """Per-core fp8 DoubleRow GEMM: [m_loc neg rows] x [n_loc nonneg rows].

    Operand roles are swapped vs the natural orientation: the nonneg side is
    the 128-wide stationary operand and the neg side is the 512-wide moving
    operand, so the matmul stream (~241ns) fully hides LDWEIGHTS (~213ns).
    Host-supplied layouts (fully contiguous per DMA):
      l{x,y}: [128, KCH, m_loc]        moving side (neg rows)
      r{x,y}: [n_ch, 128, KCH, 128]    stationary side (nonneg rows)
    """
    nc = bacc.Bacc(None)
    n_ch = n_loc // 128
    n_ms = -(-m_loc // 512)  # moving sub-tiles of <=512
    assert m_loc % 16 == 0 and n_loc % 128 == 0
    lx = nc.declare_dram_parameter("lx", [128, KCH, m_loc], FP8, isOutput=False)
    rx = nc.declare_dram_parameter("rx", [n_ch, 128, KCH, 128], FP8, isOutput=False)
    ly = nc.declare_dram_parameter("ly", [128, KCH, m_loc], FP8, isOutput=False)
    ry = nc.declare_dram_parameter("ry", [n_ch, 128, KCH, 128], FP8, isOutput=False)
    acc_out = nc.declare_dram_parameter("acc", [128, 2 * n_ch * n_ms], F32,
                                        isOutput=True)

    msizes = [min(512, m_loc - 512 * i) for i in range(n_ms)]

    with tile.TileContext(nc) as tc:
        with (
            tc.tile_pool(name="mov", bufs=1) as movp,
            tc.tile_pool(name="sta", bufs=4) as stap,
            tc.tile_pool(name="ps", bufs=4, space="PSUM") as psp,
            tc.tile_pool(name="junk", bufs=4) as junkp,
            tc.tile_pool(name="accp", bufs=1) as accp,
        ):
            acc = accp.tile([128, 2 * n_ch * n_ms], F32)
            # first GEMM block's inputs first so the PE starts early
            lt = {}
            st = {}
            lt["x"] = movp.tile([128, KCH, m_loc], FP8, tag="lx", name="lt_x")
            nc.sync.dma_start(out=lt["x"], in_=lx[:])
            st["x", 0] = stap.tile([128, KCH, 128], FP8, tag="st", name="st_x0")
            nc.sync.dma_start(out=st["x", 0], in_=rx[0])
            lt["y"] = movp.tile([128, KCH, m_loc], FP8, tag="ly", name="lt_y")
            nc.sync.dma_start(out=lt["y"], in_=ly[:])

            col = 0
            for name, rsrc in (("x", rx), ("y", ry)):
                for nch in range(n_ch):
                    if (name, nch) in st:
                        s_t = st[name, nch]
                    else:
                        s_t = stap.tile([128, KCH, 128], FP8, tag="st")
                        nc.sync.dma_start(out=s_t, in_=rsrc[nch])
                    for ms in range(n_ms):
                        msz = msizes[ms]
                        ps = psp.tile([128, 512], F32, tag="ps")
                        for kp in range(KCH // 2):
                            nc.tensor.matmul(
                                ps[:, :msz],
                                lhsT=s_t[:, 2 * kp:2 * kp + 2, :],
                                rhs=lt[name][:, 2 * kp:2 * kp + 2,
                                             512 * ms:512 * ms + msz],
                                start=(kp == 0), stop=(kp == KCH // 2 - 1),
                                perf_mode=mybir.MatmulPerfMode.DoubleRow)
                        j = junkp.tile([128, 512], BF16, tag="junk")
                        nc.scalar.activation(
                            j[:, :msz], ps[:, :msz],
                            mybir.ActivationFunctionType.Exp,
                            scale=1.0 / (TAU * FP8_SCALE * FP8_SCALE),
                            accum_out=acc[:, col:col + 1])
                        col += 1
            nc.sync.dma_start(out=acc_out[:], in_=acc)
    nc.compile()
    return nc


def _run_spmd(key, builder, in_maps):
    import os
    if key not in _CACHE:
        _CACHE[key] = builder()
    nc = _CACHE[key]
    trace = bool(os.environ.get("COCOA_TRACE"))
    res = run_bass_kernel_spmd(nc, in_maps, list(range(NCORES)), trace=trace)
    LAST_RESULTS.append((key, res))
    return res.results


def kernel(x_pred_batch: np.ndarray, y_pred_batch: np.ndarray,
           label_batch: np.ndarray) -> np.ndarray:
    x = np.ascontiguousarray(x_pred_batch, dtype=np.float32)
    y = np.ascontiguousarray(y_pred_batch, dtype=np.float32)
    lab = np.asarray(label_batch)

    # exact mask / permutation bookkeeping on host
    zero_counts = (lab == 0).sum(axis=1)
    neg_mask = zero_counts > THRESHOLD
    idx = np.concatenate([np.flatnonzero(neg_mask), np.flatnonzero(~neg_mask)])
    n1 = int(neg_mask.sum())
    n2 = B - n1
    cnt = n1 * n2

    xp = x[idx]
    yp = y[idx]

    # ---- phase 1 ----
    in_maps = [
        {"x": xp[c * ROWS:(c + 1) * ROWS], "y": yp[c * ROWS:(c + 1) * ROWS]}
        for c in range(NCORES)
    ]
    res1 = _run_spmd("phase1", _build_phase1, in_maps)

    # pos term from per-row cos, in float64
    stats = np.stack([r["dots"] for r in res1])  # [8, 128, ngrp, 3]
    stats = stats.transpose(0, 2, 1, 3).reshape(B, 3).astype(np.float64)
    cos_pos = stats[:, 0]
    pos_error = float(np.mean(np.exp((1.0 - cos_pos) / TAU)))

    neg_total = 0.0
    if cnt > 0:
        # transposed normalized embeddings [KCH, 128, B] (permuted order)
        xt = np.concatenate([r["xt"] for r in res1], axis=2)
        yt = np.concatenate([r["yt"] for r in res1], axis=2)

        m_loc = 16 * max(1, -(-n1 // (A_SPLIT * 16)))
        n_loc = 128 * max(1, -(-n2 // (B_SPLIT * 128)))
        n1p, n2p = A_SPLIT * m_loc, B_SPLIT * n_loc
        n_ch = n_loc // 128
        n_ms = -(-m_loc // 512)

        padded = {}
        for nm, t in (("x", xt), ("y", yt)):
            t = (t.astype(np.float32) * FP8_SCALE).astype(FP8_NP)
            lhs = np.zeros((KCH, 128, n1p), FP8_NP)
            lhs[:, :, :n1] = t[:, :, :n1]
            rhs = np.zeros((KCH, 128, n2p), FP8_NP)
            rhs[:, :, :n2] = t[:, :, n1:]
            # swizzle to fully-contiguous per-DMA layouts (see _build_phase2)
            padded["l" + nm] = np.ascontiguousarray(lhs.transpose(1, 0, 2))
            padded["r" + nm] = np.ascontiguousarray(
                rhs.reshape(KCH, 128, B_SPLIT * n_ch, 128).transpose(2, 1, 0, 3))

        in_maps2 = []
        for c in range(NCORES):
            a, bgrid = divmod(c, B_SPLIT)
            cmap = {}
            for nm in ("x", "y"):
                cmap["l" + nm] = np.ascontiguousarray(
                    padded["l" + nm][:, :, a * m_loc:(a + 1) * m_loc])
                cmap["r" + nm] = padded["r" + nm][bgrid * n_ch:(bgrid + 1) * n_ch]
            in_maps2.append(cmap)

        res2 = _run_spmd(("phase2v2", m_loc, n_loc), lambda: _build_phase2(m_loc, n_loc),
                         in_maps2)

        n_half = n_ch * n_ms
        sx = sy = 0.0
        for r in res2:
            acc = r["acc"].astype(np.float64)
            sx += acc[:, :n_half].sum()
            sy += acc[:, n_half:].sum()
        pad = float(n1p) * n2p - float(n1) * n2
        neg_total = ((sx - pad) + (sy - pad)) / cnt

    return np.float32(pos_error + neg_total)



# revision 3
# speedup vs baseline: 2.2599x; 2.2599x over previous
"""Trainium2 Bass kernel for the Cocoa contrastive loss.

loss = mean_i exp((1 - cos(x_i, y_i))/tau)
     + sum_{i in neg, j not in neg} exp(cos(x_i, x_j)/tau) / cnt
     + sum_{i in neg, j not in neg} exp(cos(y_i, y_j)/tau) / cnt

with neg = rows whose label has > 32 zeros, cnt = n_neg * n_nonneg.

Strategy (8 NeuronCores):
  Host: neg mask (exact integer math), l2-normalize in f32, pos term in
        float64, fp8(e4m3) quantization and the transposed [D, rows]
        layouts the GEMM wants.  All O(B*D) work; the O(B^2*D) part runs
        on the device.
  Device (single SPMD launch, 4x2 grid over neg x nonneg rows): per core
        a [m_loc x n_loc] slab of each Gram sim = Z_neg @ Z_nonneg^T with
        K=D on partitions, fp8 DoubleRow matmuls (the 157 TF/s sustained
        peak; the matmul stream fully hides LDWEIGHTS), exp(sim/tau) on
        ScalarE with per-partition accumulation into a [128, 2*n_ch]
        column vector per core.
        Inputs are loaded in many small DMAs issued in first-use order so
        the PE starts as soon as the first k-chunks land instead of
        waiting for the full moving tile.
  Host: combine partial sums (subtract the exp(0)=1 contributions of the
        zero padding), add the pos term.
"""

import numpy as np
import ml_dtypes

import concourse.bass as bass
import concourse.bacc as bacc
import concourse.mybir as mybir
import concourse.tile as tile
from concourse.bass_utils import run_bass_kernel_spmd

TAU = 0.1
THRESHOLD = 32
B, D, L = 4096, 4096, 64
NCORES = 8
KCH = D // 128      # 32 contraction chunks of 128
KSP = KCH // 2      # 16 matmul steps (DoubleRow: 2 chunks per matmul)
A_SPLIT, B_SPLIT = 4, 2  # core grid over (neg rows, nonneg rows)

F32 = mybir.dt.float32
BF16 = mybir.dt.bfloat16
FP8 = mybir.dt.float8e4
FP8_NP = ml_dtypes.float8_e4m3fn
FP8_SCALE = 24.0  # centers N(0, 1/4096) values in e4m3's normal range

# module-level caches so repeated kernel() calls don't rebuild/recompile
_CACHE: dict = {}

# filled in by the last kernel() call when tracing is enabled (test harness use)
LAST_RESULTS: list = []


def _build_gram(m_loc: int, n_ch: int) -> bass.Bass:
    """Per-core fp8 DoubleRow GEMM: [m_loc neg rows] x [n_ch*128 nonneg rows].

    The nonneg side is the 128-wide stationary operand, the neg side the
    m_loc-wide moving operand, so the matmul stream hides LDWEIGHTS.
    Host-supplied layouts (each DMA below is fully contiguous in DRAM):
      l{x,y}: [KSP, 128, 2, m_loc]      moving side (neg rows)
      r{x,y}: [n_ch, 2, 128, KSP, 128]  stationary side (nonneg rows)
    """
    nc = bacc.Bacc(None)
    lx = nc.declare_dram_parameter("lx", [KSP, 128, 2, m_loc], FP8, isOutput=False)
    ly = nc.declare_dram_parameter("ly", [KSP, 128, 2, m_loc], FP8, isOutput=False)
    rx = nc.declare_dram_parameter("rx", [n_ch, 2, 128, KSP, 128], FP8, isOutput=False)
    ry = nc.declare_dram_parameter("ry", [n_ch, 2, 128, KSP, 128], FP8, isOutput=False)
    acc_out = nc.declare_dram_parameter("acc", [128, 2 * n_ch], F32, isOutput=True)

    with tile.TileContext(nc) as tc:
        with (
            tc.tile_pool(name="mov", bufs=1) as movp,
            tc.tile_pool(name="sta", bufs=1) as stap,
            tc.tile_pool(name="ps", bufs=4, space="PSUM") as psp,
            tc.tile_pool(name="junk", bufs=4) as junkp,
            tc.tile_pool(name="accp", bufs=1) as accp,
        ):
            acc = accp.tile([128, 2 * n_ch], F32)
            lt = {
                "x": movp.tile([128, KSP, 2, m_loc], FP8, tag="ltx", name="lt_x"),
                "y": movp.tile([128, KSP, 2, m_loc], FP8, tag="lty", name="lt_y"),
            }
            # all stationary tiles live for the whole kernel (distinct tags:
            # no buffer rotation): SBUF is big enough and this avoids
            # pool-reuse false dependencies on the loads
            st = {
                (nm, c): stap.tile([128, KCH, 128], FP8, tag=f"st{nm}{c}",
                                   name=f"st_{nm}{c}")
                for nm in ("x", "y")
                for c in range(n_ch)
            }

            def load_st(nm, src, c, half):
                nc.sync.dma_start(
                    out=st[nm, c][:, 16 * half:16 * half + 16, :],
                    in_=src[c, half],
                )

            # DMA issue in first-use order: the first matmul needs lx k0 +
            # st_x0's first half; the rest of lx and st_x0 are consumed over
            # tile 0's ~3.1us stream; each later stationary tile every
            # ~3.1us; lt_y not until half way.
            nc.sync.dma_start(out=lt["x"][:, 0], in_=lx[0])
            load_st("x", rx, 0, 0)
            nc.sync.dma_start(out=lt["x"][:, 1], in_=lx[1])
            load_st("x", rx, 0, 1)
            for k in range(2, KSP):
                nc.sync.dma_start(out=lt["x"][:, k], in_=lx[k])
            for c in range(1, n_ch):
                load_st("x", rx, c, 0)
                load_st("x", rx, c, 1)
            for k in range(KSP):
                nc.sync.dma_start(out=lt["y"][:, k], in_=ly[k])
            for c in range(n_ch):
                load_st("y", ry, c, 0)
                load_st("y", ry, c, 1)

            col = 0
            for nm in ("x", "y"):
                for c in range(n_ch):
                    s_t = st[nm, c]
                    ps = psp.tile([128, m_loc], F32, tag="ps")
                    for kp in range(KSP):
                        nc.tensor.matmul(
                            ps,
                            lhsT=s_t[:, 2 * kp:2 * kp + 2, :],
                            rhs=lt[nm][:, kp, :, :],
                            start=(kp == 0), stop=(kp == KSP - 1),
                            perf_mode=mybir.MatmulPerfMode.DoubleRow)
                    j = junkp.tile([128, m_loc], BF16, tag="junk")
                    nc.scalar.activation(
                        j, ps,
                        mybir.ActivationFunctionType.Exp,
                        scale=1.0 / (TAU * FP8_SCALE * FP8_SCALE),
                        accum_out=acc[:, col:col + 1])
                    col += 1
            nc.sync.dma_start(out=acc_out[:], in_=acc)
    nc.compile()
    return nc


def _run_spmd(key, builder, in_maps):
    import os
    if key not in _CACHE:
        _CACHE[key] = builder()
    nc = _CACHE[key]
    trace = bool(os.environ.get("COCOA_TRACE"))
    res = run_bass_kernel_spmd(nc, in_maps, list(range(NCORES)), trace=trace)
    LAST_RESULTS.append((key, res))
    return res.results


def kernel(x_pred_batch: np.ndarray, y_pred_batch: np.ndarray,
           label_batch: np.ndarray) -> np.ndarray:
    x = np.ascontiguousarray(x_pred_batch, dtype=np.float32)
    y = np.ascontiguousarray(y_pred_batch, dtype=np.float32)
    lab = np.asarray(label_batch)

    # exact mask on host
    zero_counts = (lab == 0).sum(axis=1)
    neg_mask = zero_counts > THRESHOLD
    n1 = int(neg_mask.sum())
    n2 = B - n1
    cnt = n1 * n2

    # l2-normalize; pos term in float64
    xn = x / np.linalg.norm(x, axis=1, keepdims=True)
    yn = y / np.linalg.norm(y, axis=1, keepdims=True)
    cos_pos = np.einsum("ij,ij->i", xn.astype(np.float64), yn.astype(np.float64))
    pos_error = float(np.mean(np.exp((1.0 - cos_pos) / TAU)))

    if cnt == 0:
        return np.float32(pos_error)

    m_loc = 16 * max(1, -(-n1 // (A_SPLIT * 16)))
    n_loc = 128 * max(1, -(-n2 // (B_SPLIT * 128)))
    n1p, n2p = A_SPLIT * m_loc, B_SPLIT * n_loc
    n_ch = n_loc // 128
    n_ch_tot = B_SPLIT * n_ch

    padded = {}
    for nm, zn in (("x", xn), ("y", yn)):
        q = (zn * FP8_SCALE).astype(FP8_NP)  # [B, D]
        lhs = np.zeros((D, n1p), FP8_NP)
        lhs[:, :n1] = q[neg_mask].T
        rhs = np.zeros((D, n2p), FP8_NP)
        rhs[:, :n2] = q[~neg_mask].T
        # moving: [KSP, 128, 2, n1p]; element (k, p, r, m) = Z[(2k+r)*128+p, m]
        padded["l" + nm] = np.ascontiguousarray(
            lhs.reshape(KSP, 2, 128, n1p).transpose(0, 2, 1, 3))
        # stationary: [n_ch_tot, 2, 128, KSP, 128];
        # element (c, h, p, k, j) = Z[(16h+k)*128+p, 128c+j]
        padded["r" + nm] = np.ascontiguousarray(
            rhs.reshape(2, KSP, 128, n_ch_tot, 128).transpose(3, 0, 2, 1, 4))

    in_maps = []
    for c in range(NCORES):
        a, bgrid = divmod(c, B_SPLIT)
        cmap = {}
        for nm in ("x", "y"):
            cmap["l" + nm] = np.ascontiguousarray(
                padded["l" + nm][:, :, :, a * m_loc:(a + 1) * m_loc])
            cmap["r" + nm] = padded["r" + nm][bgrid * n_ch:(bgrid + 1) * n_ch]
        in_maps.append(cmap)

    res = _run_spmd(("gram", m_loc, n_ch), lambda: _build_gram(m_loc, n_ch),
                    in_maps)

    sx = sy = 0.0
    for r in res:
        acc = r["acc"].astype(np.float64)
        sx += acc[:, :n_ch].sum()
        sy += acc[:, n_ch:].sum()
    pad = float(n1p) * n2p - float(n1) * n2
    neg_total = ((sx - pad) + (sy - pad)) / cnt

    return np.float32(pos_error + neg_total)


# revision 6
# speedup vs baseline: 2.3025x; 1.0188x over previous
"""Trainium2 Bass kernel for the Cocoa contrastive loss.

loss = mean_i exp((1 - cos(x_i, y_i))/tau)
     + sum_{i in neg, j not in neg} exp(cos(x_i, x_j)/tau) / cnt
     + sum_{i in neg, j not in neg} exp(cos(y_i, y_j)/tau) / cnt

with neg = rows whose label has > 32 zeros, cnt = n_neg * n_nonneg.

Strategy (8 NeuronCores):
  Host: neg mask (exact integer math), l2-normalize in f32, pos term in
        float64, fp8(e4m3) quantization and the transposed [D, rows]
        layouts the GEMM wants.  All O(B*D) work; the O(B^2*D) part runs
        on the device.
  Device (single SPMD launch, 4x2 grid over neg x nonneg rows): per core
        a [m_loc x n_loc] slab of each Gram sim = Z_neg @ Z_nonneg^T with
        K=D on partitions, fp8 DoubleRow matmuls (the 157 TF/s sustained
        peak; the matmul stream fully hides LDWEIGHTS), exp(sim/tau) on
        ScalarE with per-partition accumulation into a [128, 2*n_ch]
        column vector per core.
        Inputs are loaded in many small DMAs issued in first-use order so
        the PE starts as soon as the first k-chunks land instead of
        waiting for the full moving tile.
  Host: combine partial sums (subtract the exp(0)=1 contributions of the
        zero padding), add the pos term.
"""

import numpy as np
import ml_dtypes

import concourse.bass as bass
import concourse.bacc as bacc
import concourse.mybir as mybir
import concourse.tile as tile
from concourse.bass_utils import run_bass_kernel_spmd

TAU = 0.1
THRESHOLD = 32
B, D, L = 4096, 4096, 64
NCORES = 8
KCH = D // 128      # 32 contraction chunks of 128
KSP = KCH // 2      # 16 matmul steps (DoubleRow: 2 chunks per matmul)
A_SPLIT, B_SPLIT = 4, 2  # core grid over (neg rows, nonneg rows)

F32 = mybir.dt.float32
BF16 = mybir.dt.bfloat16
FP8 = mybir.dt.float8e4
FP8_NP = ml_dtypes.float8_e4m3fn
FP8_SCALE = 24.0  # centers N(0, 1/4096) values in e4m3's normal range

# module-level caches so repeated kernel() calls don't rebuild/recompile
_CACHE: dict = {}

# filled in by the last kernel() call when tracing is enabled (test harness use)
LAST_RESULTS: list = []


def _build_gram(m_loc: int, n_ch: int) -> bass.Bass:
    """Per-core fp8 DoubleRow GEMM: [m_loc neg rows] x [n_ch*128 nonneg rows].

    The nonneg side is the 128-wide stationary operand, the neg side the
    m_loc-wide moving operand, so the matmul stream hides LDWEIGHTS.
    Host-supplied layouts (each DMA below is fully contiguous in DRAM):
      l{x,y}: [KSP, 128, 2, m_loc]      moving side (neg rows)
      r{x,y}: [n_ch, 2, 128, KSP, 128]  stationary side (nonneg rows)
    """
    nc = bacc.Bacc(None)
    lx = nc.declare_dram_parameter("lx", [KSP, 128, 2, m_loc], FP8, isOutput=False)
    ly = nc.declare_dram_parameter("ly", [KSP, 128, 2, m_loc], FP8, isOutput=False)
    rx = nc.declare_dram_parameter("rx", [n_ch, 2, 128, KSP, 128], FP8, isOutput=False)
    ry = nc.declare_dram_parameter("ry", [n_ch, 2, 128, KSP, 128], FP8, isOutput=False)
    # last column is warmup junk (host ignores it)
    acc_out = nc.declare_dram_parameter("acc", [128, 2 * n_ch + 1], F32,
                                        isOutput=True)

    with tile.TileContext(nc) as tc:
        with (
            tc.tile_pool(name="mov", bufs=1) as movp,
            tc.tile_pool(name="sta", bufs=1) as stap,
            tc.tile_pool(name="ps", bufs=4, space="PSUM") as psp,
            tc.tile_pool(name="junk", bufs=4) as junkp,
            tc.tile_pool(name="accp", bufs=1) as accp,
        ):
            acc = accp.tile([128, 2 * n_ch + 1], F32)
            lt = {
                "x": movp.tile([128, KSP, 2, m_loc], FP8, tag="ltx", name="lt_x"),
                "y": movp.tile([128, KSP, 2, m_loc], FP8, tag="lty", name="lt_y"),
            }
            # all stationary tiles live for the whole kernel (distinct tags:
            # no buffer rotation): SBUF is big enough and this avoids
            # pool-reuse false dependencies on the loads
            st = {
                (nm, c): stap.tile([128, KCH, 128], FP8, tag=f"st{nm}{c}",
                                   name=f"st_{nm}{c}")
                for nm in ("x", "y")
                for c in range(n_ch)
            }

            def load_st(nm, src, c, half):
                nc.sync.dma_start(
                    out=st[nm, c][:, 16 * half:16 * half + 16, :],
                    in_=src[c, half],
                )

            # DMA issue in first-use order: the first matmul needs lx k0 +
            # st_x0's first half; the rest of lx and st_x0 are consumed over
            # tile 0's ~3.1us stream; each later stationary tile every
            # ~3.1us; lt_y not until half way.
            nc.sync.dma_start(out=lt["x"][:, 0], in_=lx[0])
            load_st("x", rx, 0, 0)
            nc.sync.dma_start(out=lt["x"][:, 1], in_=lx[1])
            load_st("x", rx, 0, 1)
            for k in range(2, KSP):
                nc.sync.dma_start(out=lt["x"][:, k], in_=lx[k])
            for c in range(1, n_ch):
                load_st("x", rx, c, 0)
                load_st("x", rx, c, 1)
            for k in range(KSP):
                nc.sync.dma_start(out=lt["y"][:, k], in_=ly[k])
            for c in range(n_ch):
                load_st("y", ry, c, 0)
                load_st("y", ry, c, 1)

            # PE p-state warmup: ~3us of dummy matmuls on memset tiles while
            # the DMA prefix (lt_x + first stationary tiles) is still in
            # flight.  Without this the first ~45 real matmuls run at the
            # 0.65/1.2 GHz p-states (418/265 ns instead of 196 ns).
            wmov = movp.tile([128, 2, 512], FP8, tag="wmov", name="wmov")
            wsta = movp.tile([128, 2, 128], FP8, tag="wsta", name="wsta")
            nc.vector.memset(wmov, 0.0)
            nc.vector.memset(wsta, 0.0)
            wps = psp.tile([128, 512], F32, tag="wps")
            for _ in range(14):
                nc.tensor.matmul(
                    wps, lhsT=wsta, rhs=wmov, start=True, stop=True,
                    perf_mode=mybir.MatmulPerfMode.DoubleRow)
            wj = junkp.tile([128, 512], BF16, tag="junk")
            nc.scalar.activation(
                wj, wps, mybir.ActivationFunctionType.Exp,
                scale=1.0 / (TAU * FP8_SCALE * FP8_SCALE),
                accum_out=acc[:, 2 * n_ch:2 * n_ch + 1])

            col = 0
            for nm in ("x", "y"):
                for c in range(n_ch):
                    s_t = st[nm, c]
                    ps = psp.tile([128, m_loc], F32, tag="ps")
                    for kp in range(KSP):
                        nc.tensor.matmul(
                            ps,
                            lhsT=s_t[:, 2 * kp:2 * kp + 2, :],
                            rhs=lt[nm][:, kp, :, :],
                            start=(kp == 0), stop=(kp == KSP - 1),
                            perf_mode=mybir.MatmulPerfMode.DoubleRow)
                    j = junkp.tile([128, m_loc], BF16, tag="junk")
                    nc.scalar.activation(
                        j, ps,
                        mybir.ActivationFunctionType.Exp,
                        scale=1.0 / (TAU * FP8_SCALE * FP8_SCALE),
                        accum_out=acc[:, col:col + 1])
                    col += 1
            nc.sync.dma_start(out=acc_out[:], in_=acc)
    nc.compile()
    return nc


def _run_spmd(key, builder, in_maps):
    import os
    if key not in _CACHE:
        _CACHE[key] = builder()
    nc = _CACHE[key]
    trace = bool(os.environ.get("COCOA_TRACE"))
    res = run_bass_kernel_spmd(nc, in_maps, list(range(NCORES)), trace=trace)
    LAST_RESULTS.append((key, res))
    return res.results


def kernel(x_pred_batch: np.ndarray, y_pred_batch: np.ndarray,
           label_batch: np.ndarray) -> np.ndarray:
    x = np.ascontiguousarray(x_pred_batch, dtype=np.float32)
    y = np.ascontiguousarray(y_pred_batch, dtype=np.float32)
    lab = np.asarray(label_batch)

    # exact mask on host
    zero_counts = (lab == 0).sum(axis=1)
    neg_mask = zero_counts > THRESHOLD
    n1 = int(neg_mask.sum())
    n2 = B - n1
    cnt = n1 * n2

    # l2-normalize; pos term in float64
    xn = x / np.linalg.norm(x, axis=1, keepdims=True)
    yn = y / np.linalg.norm(y, axis=1, keepdims=True)
    cos_pos = np.einsum("ij,ij->i", xn.astype(np.float64), yn.astype(np.float64))
    pos_error = float(np.mean(np.exp((1.0 - cos_pos) / TAU)))

    if cnt == 0:
        return np.float32(pos_error)

    m_loc = 16 * max(1, -(-n1 // (A_SPLIT * 16)))
    n_loc = 128 * max(1, -(-n2 // (B_SPLIT * 128)))
    n1p, n2p = A_SPLIT * m_loc, B_SPLIT * n_loc
    n_ch = n_loc // 128
    n_ch_tot = B_SPLIT * n_ch

    padded = {}
    for nm, zn in (("x", xn), ("y", yn)):
        q = (zn * FP8_SCALE).astype(FP8_NP)  # [B, D]
        lhs = np.zeros((D, n1p), FP8_NP)
        lhs[:, :n1] = q[neg_mask].T
        rhs = np.zeros((D, n2p), FP8_NP)
        rhs[:, :n2] = q[~neg_mask].T
        # moving: [KSP, 128, 2, n1p]; element (k, p, r, m) = Z[(2k+r)*128+p, m]
        padded["l" + nm] = np.ascontiguousarray(
            lhs.reshape(KSP, 2, 128, n1p).transpose(0, 2, 1, 3))
        # stationary: [n_ch_tot, 2, 128, KSP, 128];
        # element (c, h, p, k, j) = Z[(16h+k)*128+p, 128c+j]
        padded["r" + nm] = np.ascontiguousarray(
            rhs.reshape(2, KSP, 128, n_ch_tot, 128).transpose(3, 0, 2, 1, 4))

    in_maps = []
    for c in range(NCORES):
        a, bgrid = divmod(c, B_SPLIT)
        cmap = {}
        for nm in ("x", "y"):
            cmap["l" + nm] = np.ascontiguousarray(
                padded["l" + nm][:, :, :, a * m_loc:(a + 1) * m_loc])
            cmap["r" + nm] = padded["r" + nm][bgrid * n_ch:(bgrid + 1) * n_ch]
        in_maps.append(cmap)

    res = _run_spmd(("gram", m_loc, n_ch), lambda: _build_gram(m_loc, n_ch),
                    in_maps)

    sx = sy = 0.0
    for r in res:
        acc = r["acc"].astype(np.float64)
        sx += acc[:, :n_ch].sum()
        sy += acc[:, n_ch:].sum()
    pad = float(n1p) * n2p - float(n1) * n2
    neg_total = ((sx - pad) + (sy - pad)) / cnt

    return np.float32(pos_error + neg_total)


# revision 12
# speedup vs baseline: 2.4308x; 1.0557x over previous
"""Trainium2 Bass kernel for the Cocoa contrastive loss.

loss = mean_i exp((1 - cos(x_i, y_i))/tau)
     + sum_{i in neg, j not in neg} exp(cos(x_i, x_j)/tau) / cnt
     + sum_{i in neg, j not in neg} exp(cos(y_i, y_j)/tau) / cnt

with neg = rows whose label has > 32 zeros, cnt = n_neg * n_nonneg.

Strategy (8 NeuronCores):
  Host: neg mask (exact integer math), l2-normalize in f32, pos term in
        float64, fp8(e4m3) quantization and the transposed [D, rows]
        layouts the GEMM wants.  All O(B*D) work; the O(B^2*D) part runs
        on the device.
  Device (single SPMD launch, 4x2 grid over neg x nonneg rows): per core
        a [m_loc x n_loc] slab of each Gram sim = Z_neg @ Z_nonneg^T with
        K=D on partitions, fp8 DoubleRow matmuls (the 157 TF/s sustained
        peak; the matmul stream fully hides LDWEIGHTS), exp(sim/tau) on
        ScalarE with per-partition accumulation into a [128, 2*n_ch]
        column vector per core.
        Inputs are loaded in many small DMAs issued in first-use order so
        the PE starts as soon as the first k-chunks land instead of
        waiting for the full moving tile.
  Host: combine partial sums (subtract the exp(0)=1 contributions of the
        zero padding), add the pos term.
"""

import numpy as np
import ml_dtypes

import concourse.bass as bass
import concourse.bacc as bacc
import concourse.mybir as mybir
import concourse.tile as tile
from concourse.bass_utils import run_bass_kernel_spmd

TAU = 0.1
THRESHOLD = 32
B, D, L = 4096, 4096, 64
NCORES = 8
KCH = D // 128      # 32 contraction chunks of 128
KSP = KCH // 2      # 16 matmul steps (DoubleRow: 2 chunks per matmul)
A_SPLIT, B_SPLIT = 4, 2  # core grid over (neg rows, nonneg rows)

F32 = mybir.dt.float32
BF16 = mybir.dt.bfloat16
FP8 = mybir.dt.float8e4
FP8_NP = ml_dtypes.float8_e4m3fn
FP8_SCALE = 24.0  # centers N(0, 1/4096) values in e4m3's normal range

# module-level caches so repeated kernel() calls don't rebuild/recompile
_CACHE: dict = {}

# filled in by the last kernel() call when tracing is enabled (test harness use)
LAST_RESULTS: list = []


def _build_gram(m_loc: int, n_ch: int) -> bass.Bass:
    """Per-core fp8 DoubleRow GEMM: [m_loc neg rows] x [n_ch*128 nonneg rows].

    The nonneg side is the 128-wide stationary operand, the neg side the
    m_loc-wide moving operand, so the matmul stream hides LDWEIGHTS.
    Each dma_start costs ~650ns of serial Sync-sequencer descriptor time,
    so loads are batched into few triggers: 1 per stationary tile, 4 for
    the first moving tile (so the PE can start on the first k-quarter),
    1 for the second moving tile.
    Host-supplied layouts:
      l{x,y}: [128, KSP, 2, m_loc]   moving side (neg rows)
      r{x,y}: [n_ch, 128, KCH, 128]  stationary side (nonneg rows)
    """
    nc = bacc.Bacc(None)
    lx = nc.declare_dram_parameter("lx", [128, KSP, 2, m_loc], FP8, isOutput=False)
    ly = nc.declare_dram_parameter("ly", [128, KSP, 2, m_loc], FP8, isOutput=False)
    rx = nc.declare_dram_parameter("rx", [n_ch, 128, KCH, 128], FP8, isOutput=False)
    ry = nc.declare_dram_parameter("ry", [n_ch, 128, KCH, 128], FP8, isOutput=False)
    # last column is warmup junk (host ignores it)
    acc_out = nc.declare_dram_parameter("acc", [128, 2 * n_ch + 1], F32,
                                        isOutput=True)

    with tile.TileContext(nc) as tc:
        with (
            tc.tile_pool(name="mov", bufs=1) as movp,
            tc.tile_pool(name="sta", bufs=1) as stap,
            tc.tile_pool(name="ps", bufs=4, space="PSUM") as psp,
            tc.tile_pool(name="junk", bufs=4) as junkp,
            tc.tile_pool(name="accp", bufs=1) as accp,
        ):
            acc = accp.tile([128, 2 * n_ch + 1], F32)
            lt = {
                "x": movp.tile([128, KSP, 2, m_loc], FP8, tag="ltx", name="lt_x"),
                "y": movp.tile([128, KSP, 2, m_loc], FP8, tag="lty", name="lt_y"),
            }
            # all stationary tiles live for the whole kernel (distinct tags:
            # no buffer rotation): SBUF is big enough and this avoids
            # pool-reuse false dependencies on the loads
            st = {
                (nm, c): stap.tile([128, KCH, 128], FP8, tag=f"st{nm}{c}",
                                   name=f"st_{nm}{c}")
                for nm in ("x", "y")
                for c in range(n_ch)
            }

            # DMA issue in first-use order.  Tile 0 needs st_x0 + lt_x; lt_x
            # is split in 4 k-quarters so its first matmuls can start after
            # ~1MB instead of 2.5MB.  Everything else is single-trigger.
            nc.sync.dma_start(out=st["x", 0], in_=rx[0])
            for g in range(4):
                nc.sync.dma_start(out=lt["x"][:, 4 * g:4 * g + 4],
                                  in_=lx[:, 4 * g:4 * g + 4])
            for c in range(1, n_ch):
                nc.sync.dma_start(out=st["x", c], in_=rx[c])
            nc.sync.dma_start(out=lt["y"], in_=ly[:])
            for c in range(n_ch):
                nc.sync.dma_start(out=st["y", c], in_=ry[c])

            # PE p-state warmup: ~3us of dummy matmuls on memset tiles while
            # the DMA prefix (lt_x + first stationary tiles) is still in
            # flight.  Without this the first ~45 real matmuls run at the
            # 0.65/1.2 GHz p-states (418/265 ns instead of 196 ns).
            wmov = movp.tile([128, 2, 192], FP8, tag="wmov", name="wmov")
            wsta = movp.tile([128, 2, 128], FP8, tag="wsta", name="wsta")
            nc.vector.memset(wmov, 0.0)
            nc.vector.memset(wsta, 0.0)
            wps = psp.tile([128, 192], F32, tag="wps")
            for _ in range(34):
                nc.tensor.matmul(
                    wps, lhsT=wsta, rhs=wmov, start=True, stop=True,
                    perf_mode=mybir.MatmulPerfMode.DoubleRow)
            wj = junkp.tile([128, 192], BF16, tag="junk")
            nc.scalar.activation(
                wj, wps, mybir.ActivationFunctionType.Exp,
                scale=1.0 / (TAU * FP8_SCALE * FP8_SCALE),
                accum_out=acc[:, 2 * n_ch:2 * n_ch + 1])

            col = 0
            for nm in ("x", "y"):
                for c in range(n_ch):
                    s_t = st[nm, c]
                    ps = psp.tile([128, m_loc], F32, tag="ps")
                    for kp in range(KSP):
                        nc.tensor.matmul(
                            ps,
                            lhsT=s_t[:, 2 * kp:2 * kp + 2, :],
                            rhs=lt[nm][:, kp, :, :],
                            start=(kp == 0), stop=(kp == KSP - 1),
                            perf_mode=mybir.MatmulPerfMode.DoubleRow)
                    j = junkp.tile([128, m_loc], BF16, tag="junk")
                    nc.scalar.activation(
                        j, ps,
                        mybir.ActivationFunctionType.Exp,
                        scale=1.0 / (TAU * FP8_SCALE * FP8_SCALE),
                        accum_out=acc[:, col:col + 1])
                    col += 1
            nc.sync.dma_start(out=acc_out[:], in_=acc)
    nc.compile()
    return nc


def _run_spmd(key, builder, in_maps):
    import os
    if key not in _CACHE:
        _CACHE[key] = builder()
    nc = _CACHE[key]
    trace = bool(os.environ.get("COCOA_TRACE"))
    res = run_bass_kernel_spmd(nc, in_maps, list(range(NCORES)), trace=trace)
    LAST_RESULTS.append((key, res))
    return res.results


def kernel(x_pred_batch: np.ndarray, y_pred_batch: np.ndarray,
           label_batch: np.ndarray) -> np.ndarray:
    x = np.ascontiguousarray(x_pred_batch, dtype=np.float32)
    y = np.ascontiguousarray(y_pred_batch, dtype=np.float32)
    lab = np.asarray(label_batch)

    # exact mask on host
    zero_counts = (lab == 0).sum(axis=1)
    neg_mask = zero_counts > THRESHOLD
    n1 = int(neg_mask.sum())
    n2 = B - n1
    cnt = n1 * n2

    # l2-normalize; pos term in float64
    xn = x / np.linalg.norm(x, axis=1, keepdims=True)
    yn = y / np.linalg.norm(y, axis=1, keepdims=True)
    cos_pos = np.einsum("ij,ij->i", xn.astype(np.float64), yn.astype(np.float64))
    pos_error = float(np.mean(np.exp((1.0 - cos_pos) / TAU)))

    if cnt == 0:
        return np.float32(pos_error)

    m_loc = 16 * max(1, -(-n1 // (A_SPLIT * 16)))
    n_loc = 128 * max(1, -(-n2 // (B_SPLIT * 128)))
    n1p, n2p = A_SPLIT * m_loc, B_SPLIT * n_loc
    n_ch = n_loc // 128
    n_ch_tot = B_SPLIT * n_ch

    padded = {}
    for nm, zn in (("x", xn), ("y", yn)):
        q = (zn * FP8_SCALE).astype(FP8_NP)  # [B, D]
        lhs = np.zeros((D, n1p), FP8_NP)
        lhs[:, :n1] = q[neg_mask].T
        rhs = np.zeros((D, n2p), FP8_NP)
        rhs[:, :n2] = q[~neg_mask].T
        # moving: [128, KSP, 2, n1p]; element (p, k, r, m) = Z[(2k+r)*128+p, m]
        padded["l" + nm] = np.ascontiguousarray(
            lhs.reshape(KSP, 2, 128, n1p).transpose(2, 0, 1, 3))
        # stationary: [n_ch_tot, 128, KCH, 128];
        # element (c, p, j, i) = Z[j*128+p, 128c+i]
        padded["r" + nm] = np.ascontiguousarray(
            rhs.reshape(KCH, 128, n_ch_tot, 128).transpose(2, 1, 0, 3))

    in_maps = []
    for c in range(NCORES):
        a, bgrid = divmod(c, B_SPLIT)
        cmap = {}
        for nm in ("x", "y"):
            cmap["l" + nm] = np.ascontiguousarray(
                padded["l" + nm][..., a * m_loc:(a + 1) * m_loc])
            cmap["r" + nm] = padded["r" + nm][bgrid * n_ch:(bgrid + 1) * n_ch]
        in_maps.append(cmap)

    res = _run_spmd(("gram", m_loc, n_ch), lambda: _build_gram(m_loc, n_ch),
                    in_maps)

    sx = sy = 0.0
    for r in res:
        acc = r["acc"].astype(np.float64)
        sx += acc[:, :n_ch].sum()
        sy += acc[:, n_ch:2 * n_ch].sum()
    pad = float(n1p) * n2p - float(n1) * n2
    neg_total = ((sx - pad) + (sy - pad)) / cnt

    return np.float32(pos_error + neg_total)
